# revision 1
# baseline (speedup 1.0000x reference)
"""Single-head causal attention with RoPE on 8 trn2 NeuronCores.

B=4, T=2048, C=1024 fp32 in/out; bf16 compute (tolerance 2e-2).
Sharding: core c = (batch b = c//2, zebra-half h = c%2).  Query chunks are
interleaved 128-row blocks: core h owns global chunks {2m+h}.  Keys stay in
natural order; chunk m attends keys [0, 256*(m+1)) -- the causal profile is
identical across cores (SPMD), per-core masks arrive as data (mk tile).

Pipeline: A1 K-proj + fused in-place RoPE per [P,512] tile; A2 V-proj
sharing A1's x tiles; A3 Q-proj + RoPE -> qtd (DRAM scratch); B+C per
q-chunk: scores (per-512-slice PSUM) -> exp (no max subtraction: |S/32|<~6)
-> PE transpose -> PV accumulate -> 1/z scale -> output proj + bias -> y.
"""

import os
import sys

for _p in ("/opt/trn_rl_repo", "/root/.axon_site/_ro/trn_rl_repo"):
    if os.path.isdir(_p) and _p not in sys.path:
        sys.path.insert(0, _p)

import numpy as np
import ml_dtypes

import concourse.bass as bass
import concourse.bacc as bacc
import concourse.mybir as mybir
from concourse.tile import TileContext
from concourse.bass_utils import run_bass_kernel_spmd

f32 = mybir.dt.float32
bf16 = mybir.dt.bfloat16
AF = mybir.ActivationFunctionType
ALU = mybir.AluOpType
BF = ml_dtypes.bfloat16

B, T, C = 4, 2048, 1024
P = 128
TQ = T // 2           # queries per core
TK = T
NCH = C // P          # 8 channel chunks
NQC = TQ // P         # 8 query chunks per core
THETA = 10000.0
NEG = -1.0e9


def _ext(m):
    return 256 * (m + 1)


def _slices(n, step=512):
    out, i = [], 0
    while i < n:
        out.append((i, min(step, n - i)))
        i += step
    return out


def build_program(y_bias=True):
    nc = bacc.Bacc(None, target_bir_lowering=False)

    xkT = nc.dram_tensor("xkT", [C, TK], bf16, kind="ExternalInput")
    xqT = nc.dram_tensor("xqT", [C, TQ], bf16, kind="ExternalInput")
    wk = nc.dram_tensor("wk", [C, C], bf16, kind="ExternalInput")
    wq = nc.dram_tensor("wq", [C, C], bf16, kind="ExternalInput")
    wv = nc.dram_tensor("wv", [C, C], bf16, kind="ExternalInput")
    wo = nc.dram_tensor("wo", [C, C], bf16, kind="ExternalInput")
    bq2 = nc.dram_tensor("bq2", [P, NCH], f32, kind="ExternalInput")
    bk2 = nc.dram_tensor("bk2", [P, NCH], f32, kind="ExternalInput")
    bor = nc.dram_tensor("bor", [1, C], bf16, kind="ExternalInput")  # bv@Wo+bo
    one1 = nc.dram_tensor("one1", [1, P], bf16, kind="ExternalInput")
    coskT = nc.dram_tensor("coskT", [C // 2, TK], bf16, kind="ExternalInput")
    sinkT = nc.dram_tensor("sinkT", [C // 2, TK], bf16, kind="ExternalInput")
    cosqT = nc.dram_tensor("cosqT", [C // 2, TQ], bf16, kind="ExternalInput")
    sinqT = nc.dram_tensor("sinqT", [C // 2, TQ], bf16, kind="ExternalInput")
    mk = nc.dram_tensor("mk", [P, TK], bf16, kind="ExternalInput")
    idn = nc.dram_tensor("idn", [P, P], bf16, kind="ExternalInput")
    y = nc.dram_tensor("y", [TQ, C], bf16, kind="ExternalOutput")

    xkT3 = xkT.rearrange("(kc p) t -> kc p t", p=P)
    xqT3 = xqT.rearrange("(kc p) t -> kc p t", p=P)
    wk3 = wk.rearrange("(kc p) co -> kc p co", p=P)
    wq3 = wq.rearrange("(kc p) co -> kc p co", p=P)
    wv3 = wv.rearrange("(kc p) co -> kc p co", p=P)
    wo3 = wo.rearrange("(kc p) co -> kc p co", p=P)

    with TileContext(nc) as tc:
        with (
            tc.tile_pool(name="resid", bufs=1) as resid,
            tc.tile_pool(name="dram", bufs=1, space="DRAM") as dpool,
            tc.tile_pool(name="wgt", bufs=16) as wgtp,
            tc.tile_pool(name="xb", bufs=24) as xbp,
            tc.tile_pool(name="rope", bufs=12) as ropep,
            tc.tile_pool(name="wrk", bufs=4) as wrk,
            tc.tile_pool(name="small", bufs=4) as smallp,
            tc.tile_pool(name="psA", bufs=3, space="PSUM") as pa,
            tc.tile_pool(name="psT", bufs=1, space="PSUM") as ptr,
            tc.tile_pool(name="psO", bufs=2, space="PSUM") as po,
        ):
            # ---- constants (K-path first so A1 can start ASAP) ----
            idnt = resid.tile([P, P], bf16, name="idnt")
            nc.sync.dma_start(idnt[:], idn[:])
            bq2t = resid.tile([P, NCH], f32, name="bq2t")
            nc.sync.dma_start(bq2t[:], bq2[:])
            bk2t = resid.tile([P, NCH], f32, name="bk2t")
            nc.sync.dma_start(bk2t[:], bk2[:])

            wkc = [wgtp.tile([P, C], bf16, tag="wgt", name=f"wkc{k}")
                   for k in range(NCH)]
            wvc = [wgtp.tile([P, C], bf16, tag="wgt", name=f"wvc{k}")
                   for k in range(NCH)]
            kt = [resid.tile([P, TK], bf16, name=f"kt{i}") for i in range(NCH)]
            vt = [resid.tile([P, C], bf16, name=f"vt{j}") for j in range(TK // P)]
            qtd = dpool.tile([NCH, P, TQ], bf16, name="qtd")

            # ============ A1+A2: K-proj + RoPE, V-proj (shared x tiles) ====
            for n in range(TK // 512):
                sl = slice(n * 512, (n + 1) * 512)
                xts = []
                for k in range(NCH):
                    xt_ = xbp.tile([P, 512], bf16, tag="xb")
                    nc.sync.dma_start(xt_[:], xkT3[k, :, sl])
                    xts.append(xt_)

                def k_pair_rope(i, ri, rj, sl=sl):
                    """NeoX-pair RoPE: chunks (i, i+4) rotate into kt from
                    raw projections ri/rj; all-bf16 SBUF ops, no PE."""
                    j = i + 4
                    cs = ropep.tile([P, 512], bf16, tag="cs")
                    nc.sync.dma_start(cs[:], coskT[i * P:(i + 1) * P, sl])
                    sn = ropep.tile([P, 512], bf16, tag="sn")
                    nc.sync.dma_start(sn[:], sinkT[i * P:(i + 1) * P, sl])
                    t1 = wrk.tile([P, 512], bf16, tag="rot")
                    t2 = wrk.tile([P, 512], bf16, tag="rot")
                    nc.vector.tensor_tensor(t1[:], rj[:], sn[:], ALU.mult)
                    nc.vector.tensor_tensor(t2[:], ri[:], sn[:], ALU.mult)
                    nc.vector.tensor_tensor(kt[i][:, sl], ri[:], cs[:],
                                            ALU.mult)
                    nc.vector.tensor_tensor(kt[i][:, sl], kt[i][:, sl], t1[:],
                                            ALU.subtract)
                    nc.vector.tensor_tensor(kt[j][:, sl], rj[:], cs[:],
                                            ALU.mult)
                    nc.vector.tensor_tensor(kt[j][:, sl], kt[j][:, sl], t2[:],
                                            ALU.add)

                if n == 0:
                    for k in range(NCH):
                        nc.sync.dma_start(wkc[k][:], wk3[k])
                    for k in range(NCH):
                        nc.sync.dma_start(wvc[k][:], wv3[k])
                # K-proj in partner order; rope fires when a pair completes
                raw = {}
                for i in (0, 4, 1, 5, 2, 6, 3, 7):
                    ps = pa.tile([P, 512], f32, tag="pa")
                    for k in range(NCH):
                        nc.tensor.matmul(ps[:], wkc[k][:, i * P:(i + 1) * P],
                                         xts[k][:],
                                         start=(k == 0), stop=(k == NCH - 1))
                    ri = wrk.tile([P, 512], bf16, tag="qraw")
                    nc.scalar.activation(ri[:], ps[:], AF.Identity,
                                         bias=bk2t[:, i:i + 1], scale=1.0)
                    raw[i] = ri
                    if i >= 4:
                        k_pair_rope(i - 4, raw.pop(i - 4), raw.pop(i))
                if n == 3:
                    wqc = [wgtp.tile([P, C], bf16, tag="wgt", name=f"wqc{k}")
                           for k in range(NCH)]
                    for k in range(NCH):
                        nc.sync.dma_start(wqc[k][:], wq3[k])
                # V-proj: x chunks stationary, wv moving; uses the po banks
                # (idle until phase B) so K-proj keeps pa to itself
                for ms in range(4):
                    j = n * 4 + ms
                    psv = po.tile([P, C], f32, tag="po")
                    for k in range(NCH):
                        xst = xts[k][:, ms * P:(ms + 1) * P]
                        nc.tensor.matmul(psv[:, 0:512], xst, wvc[k][:, 0:512],
                                         start=(k == 0), stop=(k == NCH - 1))
                        nc.tensor.matmul(psv[:, 512:1024], xst,
                                         wvc[k][:, 512:1024],
                                         start=(k == 0), stop=(k == NCH - 1))
                    nc.scalar.activation(vt[j][:], psv[:], AF.Copy,
                                         bias=0.0, scale=1.0)

            # ============ A3: Q-proj + RoPE -> qtd ============
            one1t = resid.tile([1, P], bf16, name="one1t")
            nc.sync.dma_start(one1t[:], one1[:])
            bort = resid.tile([1, C], bf16, name="bort")
            nc.sync.dma_start(bort[:], bor[:])
            mkt = resid.tile([P, TK], bf16, name="mkt")
            nc.sync.dma_start(mkt[:], mk[:])
            for n in range(TQ // 512):
                sl = slice(n * 512, (n + 1) * 512)
                xts = []
                for k in range(NCH):
                    xt_ = xbp.tile([P, 512], bf16, tag="xb")
                    nc.sync.dma_start(xt_[:], xqT3[k, :, sl])
                    xts.append(xt_)
                if n == 1:
                    woc = [wgtp.tile([P, C], bf16, tag="wgt", name=f"woc{k}")
                           for k in range(NCH)]
                    for k in range(NCH):
                        nc.sync.dma_start(woc[k][:], wo3[k])

                def q_pair_rope(i, ri, rj, sl=sl):
                    j = i + 4
                    cs = ropep.tile([P, 512], bf16, tag="cs")
                    nc.sync.dma_start(cs[:], cosqT[i * P:(i + 1) * P, sl])
                    sn = ropep.tile([P, 512], bf16, tag="sn")
                    nc.sync.dma_start(sn[:], sinqT[i * P:(i + 1) * P, sl])
                    t1 = wrk.tile([P, 512], bf16, tag="rot")
                    t2 = wrk.tile([P, 512], bf16, tag="rot")
                    nc.vector.tensor_tensor(t1[:], rj[:], sn[:], ALU.mult)
                    nc.vector.tensor_tensor(t2[:], ri[:], sn[:], ALU.mult)
                    nc.vector.tensor_tensor(ri[:], ri[:], cs[:], ALU.mult)
                    nc.vector.tensor_tensor(ri[:], ri[:], t1[:], ALU.subtract)
                    nc.sync.dma_start(qtd[i, :, sl], ri[:])
                    nc.vector.tensor_tensor(rj[:], rj[:], cs[:], ALU.mult)
                    nc.vector.tensor_tensor(rj[:], rj[:], t2[:], ALU.add)
                    nc.sync.dma_start(qtd[j, :, sl], rj[:])

                raw = {}
                for i in (0, 4, 1, 5, 2, 6, 3, 7):
                    ps = pa.tile([P, 512], f32, tag="pa")
                    for k in range(NCH):
                        nc.tensor.matmul(ps[:], wqc[k][:, i * P:(i + 1) * P],
                                         xts[k][:],
                                         start=(k == 0), stop=(k == NCH - 1))
                    qraw = wrk.tile([P, 512], bf16, tag="qraw")
                    nc.scalar.activation(qraw[:], ps[:], AF.Identity,
                                         bias=bq2t[:, i:i + 1], scale=1.0)
                    raw[i] = qraw
                    if i >= 4:
                        q_pair_rope(i - 4, raw.pop(i - 4), raw.pop(i))

            # ============ B+C: attention + output proj per q-chunk ============
            for m in (0, 1, 2, 3, 5, 6, 7, 4):
                ext = _ext(m)
                nS = ext // P
                sls = _slices(ext)
                qc = wrk.tile([P, NCH, P], bf16, tag="qc", name=f"qc{m}")
                nc.sync.dma_start(
                    qc[:], qtd[:, :, m * P:(m + 1) * P].rearrange("i p q -> p i q"))
                opsum = po.tile([P, C], f32, tag="po", name=f"psO{m}")
                zpart = smallp.tile([P, 4], f32, tag="zpart")
                for si, (off, w) in enumerate(sls):
                    ps = pa.tile([P, 512], f32, tag="pa")
                    for k in range(NCH):
                        nc.tensor.matmul(ps[:, 0:w], qc[:, k, :],
                                         kt[k][:, off:off + w],
                                         start=(k == 0), stop=(k == NCH - 1))
                    if off + w == ext:
                        nc.vector.tensor_tensor(
                            ps[:, w - 256:w], ps[:, w - 256:w],
                            mkt[:, ext - 256:ext], ALU.add)
                    es = wrk.tile([P, 512], bf16, tag="es")
                    nc.scalar.activation(es[:, 0:w], ps[:, 0:w], AF.Exp,
                                         bias=0.0, scale=1.0 / 32.0,
                                         accum_out=zpart[:, si:si + 1])
                    pt = ptr.tile([P, 1024], bf16, tag="ptr")
                    for jj in range(w // P):
                        nc.tensor.transpose(pt[:, jj * P:(jj + 1) * P],
                                            es[:, jj * P:(jj + 1) * P], idnt[:])
                    et = wrk.tile([P, 512], bf16, tag="et")
                    nc.vector.tensor_copy(et[:, 0:w], pt[:, 0:w])
                    for jj in range(w // P):
                        j = off // P + jj
                        for ch in range(2):
                            nc.tensor.matmul(
                                opsum[:, ch * 512:(ch + 1) * 512],
                                et[:, jj * P:(jj + 1) * P],
                                vt[j][:, ch * 512:(ch + 1) * 512],
                                start=(j == 0), stop=(j == nS - 1))
                z = smallp.tile([P, 1], f32, tag="z")
                nc.vector.tensor_reduce(z[:], zpart[:, 0:len(sls)],
                                        axis=mybir.AxisListType.X, op=ALU.add)
                zinv = smallp.tile([P, 1], f32, tag="zinv")
                nc.vector.reciprocal(zinv[:], z[:])
                osb = wrk.tile([P, C], bf16, tag="osb")
                nc.scalar.activation(osb[:, 0:512], opsum[:, 0:512], AF.Copy,
                                     bias=0.0, scale=zinv[:])
                nc.scalar.activation(osb[:, 512:1024], opsum[:, 512:1024],
                                     AF.Copy, bias=0.0, scale=zinv[:])
                # ---- C: Y(m) = osb @ Wo + bor ----
                yps = po.tile([P, C], f32, tag="po", name=f"psY{m}")
                pt2 = ptr.tile([P, 1024], bf16, tag="ptr")
                ot = wrk.tile([P, 1024], bf16, tag="ot")
                for g in range(2):
                    for u in range(4):
                        k = g * 4 + u
                        nc.tensor.transpose(pt2[:, k * P:(k + 1) * P],
                                            osb[:, k * P:(k + 1) * P], idnt[:])
                    nc.vector.tensor_copy(ot[:, g * 512:(g + 1) * 512],
                                          pt2[:, g * 512:(g + 1) * 512])
                for k in range(NCH):
                    for ch in range(2):
                        nc.tensor.matmul(
                            yps[:, ch * 512:(ch + 1) * 512],
                            ot[:, k * P:(k + 1) * P],
                            woc[k][:, ch * 512:(ch + 1) * 512],
                            start=(k == 0),
                            stop=(k == NCH - 1 and not y_bias))
                if y_bias:
                    for ch in range(2):
                        nc.tensor.matmul(
                            yps[:, ch * 512:(ch + 1) * 512], one1t[:],
                            bort[0:1, ch * 512:(ch + 1) * 512],
                            start=False, stop=True)
                ysb = wrk.tile([P, C], bf16, tag="ysb")
                for q4 in range(2):
                    qs = slice(q4 * 512, (q4 + 1) * 512)
                    nc.vector.tensor_copy(ysb[:, qs], yps[:, qs])
                    nc.sync.dma_start(y[m * P:(m + 1) * P, qs], ysb[:, qs])

    nc.compile()
    return nc


def _zebra_idx(h):
    return np.concatenate(
        [np.arange(P) + (2 * m + h) * P for m in range(NQC)])


SIGMA = np.concatenate([np.arange(0, C, 2), np.arange(1, C, 2)])


def make_host_tables():
    """Half-height tables for the NeoX-pair channel layout: new channel p
    (< C/2) holds original channel 2p, p + C/2 holds 2p+1; both share
    inv_freq[p], so one [C/2, T] table serves a chunk pair."""
    inv_freq = 1.0 / (THETA ** (np.arange(0, C, 2, dtype=np.float64) / C))
    freqs = np.arange(T, dtype=np.float64)[:, None] * inv_freq[None, :]
    cos = np.cos(freqs).T.astype(np.float32)                   # [C/2, T]
    sin = np.sin(freqs).T.astype(np.float32)
    idn = np.eye(P, dtype=np.float32)
    return cos, sin, idn


def _make_mask(h):
    mkv = np.zeros((P, TK), np.float32)
    tri = np.where(np.arange(P)[:, None] >= np.arange(P)[None, :], 0.0, NEG)
    for m in range(NQC):
        base = 256 * m
        g = 2 * m + h
        for bb in range(2):
            kb = 2 * m + bb
            colsl = slice(base + bb * P, base + (bb + 1) * P)
            if kb == g:
                mkv[:, colsl] = tri
            elif kb > g:
                mkv[:, colsl] = NEG
    return mkv


def make_in_maps(x, Wq, bq, Wk, bk, Wv, bv, Wo, bo):
    cos, sin, idn = make_host_tables()
    bo2 = (bv.astype(np.float64) @ Wo.astype(np.float64) + bo).astype(np.float32)
    base = {
        "xkT": None, "xqT": None,
        "wk": np.ascontiguousarray(Wk[:, SIGMA].astype(BF)),
        "wq": np.ascontiguousarray(Wq[:, SIGMA].astype(BF)),
        "wv": np.ascontiguousarray(Wv.astype(BF)),
        "wo": np.ascontiguousarray(Wo.astype(BF)),
        "bq2": np.ascontiguousarray(
            bq[SIGMA].reshape(NCH, P).T.astype(np.float32)),
        "bk2": np.ascontiguousarray(
            bk[SIGMA].reshape(NCH, P).T.astype(np.float32)),
        "bor": bo2.reshape(1, C).astype(BF),
        "one1": np.ones((1, P), BF),
        "coskT": np.ascontiguousarray(cos.astype(BF)),
        "sinkT": np.ascontiguousarray(sin.astype(BF)),
        "idn": idn.astype(BF),
    }
    in_maps = []
    for core in range(8):
        b, h = core // 2, core % 2
        idx = _zebra_idx(h)
        m = dict(base)
        m["xkT"] = np.ascontiguousarray(x[b].T.astype(BF))
        m["xqT"] = np.ascontiguousarray(x[b][idx].T.astype(BF))
        m["cosqT"] = np.ascontiguousarray(cos[:, idx].astype(BF))
        m["sinqT"] = np.ascontiguousarray(sin[:, idx].astype(BF))
        m["mk"] = _make_mask(h).astype(BF)
        in_maps.append(m)
    return in_maps


_progs = {}


def kernel(x, Wq, bq, Wk, bk, Wv, bv, Wo, bo, _trace=False, _tracedir=None):
    x = np.ascontiguousarray(np.asarray(x, np.float32))
    args = [np.ascontiguousarray(np.asarray(a, np.float32)) for a in
            (Wq, bq, Wk, bk, Wv, bv, Wo, bo)]
    Wo_, bo_, bv_ = args[6], args[7], args[5]
    bor_val = bv_.astype(np.float64) @ Wo_.astype(np.float64) + bo_
    y_bias = bool(np.any(bor_val != 0.0))
    if y_bias not in _progs:
        _progs[y_bias] = build_program(y_bias=y_bias)
    _prog = _progs[y_bias]
    in_maps = make_in_maps(x, *args)
    kw = {}
    if _trace:
        kw = dict(trace=True, trace_cores=[0], tmpdir=_tracedir)
    res = run_bass_kernel_spmd(_prog, in_maps, core_ids=list(range(8)), **kw)
    out = np.empty((B, T, C), np.float32)
    for core in range(8):
        b, h = core // 2, core % 2
        out[b, _zebra_idx(h), :] = res.results[core]["y"].astype(np.float32)
    if _trace:
        kernel._last_results = res
    return out



# revision 6
# speedup vs baseline: 1.3138x; 1.3138x over previous
"""Single-head causal attention with RoPE on 8 trn2 NeuronCores.

B=4, T=2048, C=1024 fp32 in/out; tolerance 2e-2.
Sharding: core c = (batch b = c//2, zebra-half h = c%2).  Query chunks are
interleaved 128-row blocks: core h owns global chunks {2m+h}.  Keys stay in
natural order; chunk m attends keys [0, 256*(m+1)); per-core masks arrive
as data (mk tile).

Key optimizations over the bf16 baseline:
- Wo folded into Wv on the host (W~ = Wv@Wo): y = (P @ (x@W~))/z + bor.
  The entire output projection, its transposes and copies are gone.
- K/V~/Q projections run as fp8(e4m3) DoubleRow matmuls (2 contraction
  chunks per instruction, 0.5 cyc/col) with a 3-pass hi/lo split:
      out = x_hi@W_hi + (x_hi/64)@(64*W_lo) + x_lo@W_hi
  All passes accumulate into one PSUM chain at scale 1 (scales folded
  into host-prepared operands).  Accuracy ~ bf16 (rel err ~3e-3).
- Scores and P@V stay bf16 (fp8 there costs more vector work than it
  saves in PE time and hurts accuracy).
"""

import os
import sys

for _p in ("/opt/trn_rl_repo", "/root/.axon_site/_ro/trn_rl_repo"):
    if os.path.isdir(_p) and _p not in sys.path:
        sys.path.insert(0, _p)

import numpy as np
import ml_dtypes

import concourse.bass as bass
import concourse.bacc as bacc
import concourse.mybir as mybir
from concourse.tile import TileContext
from concourse.bass_utils import run_bass_kernel_spmd

f32 = mybir.dt.float32
bf16 = mybir.dt.bfloat16
f8 = mybir.dt.float8e4
AF = mybir.ActivationFunctionType
ALU = mybir.AluOpType
DR = mybir.MatmulPerfMode.DoubleRow
BF = ml_dtypes.bfloat16
F8 = ml_dtypes.float8_e4m3

B, T, C = 4, 2048, 1024
P = 128
TQ = T // 2           # queries per core
TK = T
NCH = C // P          # 8 channel chunks
NPR = NCH // 2        # 4 contraction chunk-pairs
NQC = TQ // P         # 8 query chunks per core
THETA = 10000.0
NEG = -1.0e9
LOSC = 64.0           # lo-part scale folded into host operands


def _ext(m):
    return 256 * (m + 1)


def _slices(n, step=512):
    out, i = [], 0
    while i < n:
        out.append((i, min(step, n - i)))
        i += step
    return out


def build_program(y_bias=True):
    nc = bacc.Bacc(None, target_bir_lowering=False)

    # x^T pair-packed: [pair j, p, 2*T] with chunk 2j in cols [:T], 2j+1 in
    # cols [T:].  hi = fp8(x), lo = fp8(x - hi) (unscaled), h64 = fp8(hi/64).
    xkh = nc.dram_tensor("xkh", [NPR, P, 2 * TK], f8, kind="ExternalInput")
    xkl = nc.dram_tensor("xkl", [NPR, P, 2 * TK], f8, kind="ExternalInput")
    xk64 = nc.dram_tensor("xk64", [NPR, P, 2 * TK], f8, kind="ExternalInput")
    xqh = nc.dram_tensor("xqh", [NPR, P, 2 * TQ], f8, kind="ExternalInput")
    xql = nc.dram_tensor("xql", [NPR, P, 2 * TQ], f8, kind="ExternalInput")
    xq64 = nc.dram_tensor("xq64", [NPR, P, 2 * TQ], f8, kind="ExternalInput")
    # weights pair-packed along input-channel rows: [pair j, p, 2*C]
    wkh = nc.dram_tensor("wkh", [NPR, P, 2 * C], f8, kind="ExternalInput")
    wkl = nc.dram_tensor("wkl", [NPR, P, 2 * C], f8, kind="ExternalInput")
    wqh = nc.dram_tensor("wqh", [NPR, P, 2 * C], f8, kind="ExternalInput")
    wql = nc.dram_tensor("wql", [NPR, P, 2 * C], f8, kind="ExternalInput")
    wvh = nc.dram_tensor("wvh", [NPR, P, 2 * C], f8, kind="ExternalInput")
    wvl = nc.dram_tensor("wvl", [NPR, P, 2 * C], f8, kind="ExternalInput")
    bq2 = nc.dram_tensor("bq2", [P, NCH], f32, kind="ExternalInput")
    bk2 = nc.dram_tensor("bk2", [P, NCH], f32, kind="ExternalInput")
    borf = nc.dram_tensor("borf", [P, C], bf16, kind="ExternalInput")
    coskT = nc.dram_tensor("coskT", [C // 2, TK], bf16, kind="ExternalInput")
    sinkT = nc.dram_tensor("sinkT", [C // 2, TK], bf16, kind="ExternalInput")
    cosqT = nc.dram_tensor("cosqT", [C // 2, TQ], bf16, kind="ExternalInput")
    sinqT = nc.dram_tensor("sinqT", [C // 2, TQ], bf16, kind="ExternalInput")
    mk = nc.dram_tensor("mk", [P, TK], bf16, kind="ExternalInput")
    idn = nc.dram_tensor("idn", [P, P], bf16, kind="ExternalInput")
    y = nc.dram_tensor("y", [TQ, C], bf16, kind="ExternalOutput")

    def pair2(t):
        return t.rearrange("p (two c) -> p two c", two=2)

    with TileContext(nc) as tc:
        with (
            tc.tile_pool(name="resid", bufs=1) as resid,
            tc.tile_pool(name="dram", bufs=1, space="DRAM") as dpool,
            tc.tile_pool(name="wgt", bufs=24) as wgtp,
            tc.tile_pool(name="xb", bufs=26) as xbp,
            tc.tile_pool(name="rope", bufs=12) as ropep,
            tc.tile_pool(name="wrk", bufs=4) as wrk,
            tc.tile_pool(name="small", bufs=4) as smallp,
            tc.tile_pool(name="psA", bufs=3, space="PSUM") as pa,
            tc.tile_pool(name="psT", bufs=1, space="PSUM") as ptr,
            tc.tile_pool(name="psO", bufs=2, space="PSUM") as po,
        ):
            # ---- constants (K-path first so A1 can start ASAP) ----
            idnt = resid.tile([P, P], bf16, name="idnt")
            nc.sync.dma_start(idnt[:], idn[:])
            bq2t = resid.tile([P, NCH], f32, name="bq2t")
            nc.sync.dma_start(bq2t[:], bq2[:])
            bk2t = resid.tile([P, NCH], f32, name="bk2t")
            nc.sync.dma_start(bk2t[:], bk2[:])

            wkht = [wgtp.tile([P, 2 * C], f8, tag="wgt", name=f"wkh{j}")
                    for j in range(NPR)]
            wklt = [wgtp.tile([P, 2 * C], f8, tag="wgt", name=f"wkl{j}")
                    for j in range(NPR)]
            wvht = [wgtp.tile([P, 2 * C], f8, tag="wgt", name=f"wvh{j}")
                    for j in range(NPR)]
            wvlt = [wgtp.tile([P, 2 * C], f8, tag="wgt", name=f"wvl{j}")
                    for j in range(NPR)]
            kt = [resid.tile([P, TK], bf16, name=f"kt{i}") for i in range(NCH)]
            vt = [resid.tile([P, C], bf16, name=f"vt{j}") for j in range(TK // P)]
            qtd = dpool.tile([NCH, P, TQ], bf16, name="qtd")

            # ============ A1+A2: K-proj + RoPE, V~-proj (shared x tiles) ====
            for n in range(TK // 512):
                sl = slice(n * 512, (n + 1) * 512)
                xh, xl, x64 = [], [], []
                for j in range(NPR):
                    for lst, src in ((xh, xkh), (xl, xkl), (x64, xk64)):
                        t = xbp.tile([P, 1024], f8, tag="xb")
                        nc.sync.dma_start(
                            t[:],
                            src[j].rearrange("p (two t) -> p two t",
                                             two=2)[:, :, sl])
                        lst.append(t)

                def k_pair_rope(i, ri, rj, sl=sl):
                    """NeoX-pair RoPE: chunks (i, i+4) rotate into kt."""
                    jj = i + 4
                    cs = ropep.tile([P, 512], bf16, tag="cs")
                    nc.sync.dma_start(cs[:], coskT[i * P:(i + 1) * P, sl])
                    sn = ropep.tile([P, 512], bf16, tag="sn")
                    nc.sync.dma_start(sn[:], sinkT[i * P:(i + 1) * P, sl])
                    t1 = wrk.tile([P, 512], bf16, tag="rot")
                    t2 = wrk.tile([P, 512], bf16, tag="rot")
                    nc.vector.tensor_tensor(t1[:], rj[:], sn[:], ALU.mult)
                    nc.vector.tensor_tensor(t2[:], ri[:], sn[:], ALU.mult)
                    nc.vector.tensor_tensor(kt[i][:, sl], ri[:], cs[:],
                                            ALU.mult)
                    nc.vector.tensor_tensor(kt[i][:, sl], kt[i][:, sl], t1[:],
                                            ALU.subtract)
                    nc.vector.tensor_tensor(kt[jj][:, sl], rj[:], cs[:],
                                            ALU.mult)
                    nc.vector.tensor_tensor(kt[jj][:, sl], kt[jj][:, sl],
                                            t2[:], ALU.add)

                if n == 0:
                    for j in range(NPR):
                        nc.sync.dma_start(wkht[j][:], wkh[j])
                    for j in range(NPR):
                        nc.sync.dma_start(wklt[j][:], wkl[j])
                    for j in range(NPR):
                        nc.sync.dma_start(wvht[j][:], wvh[j])
                    for j in range(NPR):
                        nc.sync.dma_start(wvlt[j][:], wvl[j])
                # K-proj in partner order; rope fires when a pair completes
                raw = {}
                for i in (0, 4, 1, 5, 2, 6, 3, 7):
                    ps = pa.tile([P, 512], f32, tag="pa")
                    csl = slice(i * P, (i + 1) * P)
                    seq = ([(wkht[j], xh[j]) for j in range(NPR)]
                           + [(wklt[j], x64[j]) for j in range(NPR)]
                           + [(wkht[j], xl[j]) for j in range(NPR)])
                    for s, (wt, xt) in enumerate(seq):
                        nc.tensor.matmul(ps[:], pair2(wt[:])[:, :, csl],
                                         pair2(xt[:]),
                                         start=(s == 0), stop=(s == len(seq) - 1),
                                         perf_mode=DR)
                    ri = wrk.tile([P, 512], bf16, tag="qraw")
                    nc.scalar.activation(ri[:], ps[:], AF.Identity,
                                         bias=bk2t[:, i:i + 1], scale=1.0)
                    raw[i] = ri
                    if i >= 4:
                        k_pair_rope(i - 4, raw.pop(i - 4), raw.pop(i))
                if n == 3:
                    wqht = [wgtp.tile([P, 2 * C], f8, tag="wgt",
                                      name=f"wqh{j}") for j in range(NPR)]
                    wqlt = [wgtp.tile([P, 2 * C], f8, tag="wgt",
                                      name=f"wql{j}") for j in range(NPR)]
                    for j in range(NPR):
                        nc.sync.dma_start(wqht[j][:], wqh[j])
                    for j in range(NPR):
                        nc.sync.dma_start(wqlt[j][:], wql[j])
                # V~-proj: x pair-chunks stationary, wv moving; po banks
                for ms in range(4):
                    jg = n * 4 + ms
                    msl = slice(ms * P, (ms + 1) * P)
                    psv = po.tile([P, C], f32, tag="po")
                    for ch in range(2):
                        chl = slice(ch * 512, (ch + 1) * 512)
                        seq = ([(xh[j], wvht[j]) for j in range(NPR)]
                               + [(x64[j], wvlt[j]) for j in range(NPR)]
                               + [(xl[j], wvht[j]) for j in range(NPR)])
                        for s, (xt, wt) in enumerate(seq):
                            nc.tensor.matmul(
                                psv[:, chl], pair2(xt[:])[:, :, msl],
                                pair2(wt[:])[:, :, chl],
                                start=(s == 0), stop=(s == len(seq) - 1),
                                perf_mode=DR)
                    nc.scalar.activation(vt[jg][:], psv[:], AF.Copy,
                                         bias=0.0, scale=1.0)

            # ============ A3: Q-proj + RoPE -> qtd ============
            mkt = resid.tile([P, TK], bf16, name="mkt")
            nc.sync.dma_start(mkt[:], mk[:])
            if y_bias:
                borft = resid.tile([P, C], bf16, name="borft")
                nc.sync.dma_start(borft[:], borf[:])
            for n in range(TQ // 512):
                sl = slice(n * 512, (n + 1) * 512)
                xh, xl, x64 = [], [], []
                for j in range(NPR):
                    for lst, src in ((xh, xqh), (xl, xql), (x64, xq64)):
                        t = xbp.tile([P, 1024], f8, tag="xb")
                        nc.sync.dma_start(
                            t[:],
                            src[j].rearrange("p (two t) -> p two t",
                                             two=2)[:, :, sl])
                        lst.append(t)

                def q_pair_rope(i, ri, rj, sl=sl):
                    jj = i + 4
                    cs = ropep.tile([P, 512], bf16, tag="cs")
                    nc.sync.dma_start(cs[:], cosqT[i * P:(i + 1) * P, sl])
                    sn = ropep.tile([P, 512], bf16, tag="sn")
                    nc.sync.dma_start(sn[:], sinqT[i * P:(i + 1) * P, sl])
                    t1 = wrk.tile([P, 512], bf16, tag="rot")
                    t2 = wrk.tile([P, 512], bf16, tag="rot")
                    nc.vector.tensor_tensor(t1[:], rj[:], sn[:], ALU.mult)
                    nc.vector.tensor_tensor(t2[:], ri[:], sn[:], ALU.mult)
                    nc.vector.tensor_tensor(ri[:], ri[:], cs[:], ALU.mult)
                    nc.vector.tensor_tensor(ri[:], ri[:], t1[:], ALU.subtract)
                    nc.sync.dma_start(qtd[i, :, sl], ri[:])
                    nc.vector.tensor_tensor(rj[:], rj[:], cs[:], ALU.mult)
                    nc.vector.tensor_tensor(rj[:], rj[:], t2[:], ALU.add)
                    nc.sync.dma_start(qtd[jj, :, sl], rj[:])

                raw = {}
                for i in (0, 4, 1, 5, 2, 6, 3, 7):
                    ps = pa.tile([P, 512], f32, tag="pa")
                    csl = slice(i * P, (i + 1) * P)
                    seq = ([(wqht[j], xh[j]) for j in range(NPR)]
                           + [(wqlt[j], x64[j]) for j in range(NPR)]
                           + [(wqht[j], xl[j]) for j in range(NPR)])
                    for s, (wt, xt) in enumerate(seq):
                        nc.tensor.matmul(ps[:], pair2(wt[:])[:, :, csl],
                                         pair2(xt[:]),
                                         start=(s == 0), stop=(s == len(seq) - 1),
                                         perf_mode=DR)
                    qraw = wrk.tile([P, 512], bf16, tag="qraw")
                    nc.scalar.activation(qraw[:], ps[:], AF.Identity,
                                         bias=bq2t[:, i:i + 1], scale=1.0)
                    raw[i] = qraw
                    if i >= 4:
                        q_pair_rope(i - 4, raw.pop(i - 4), raw.pop(i))

            # ============ B: attention -> y per q-chunk ============
            for m in (0, 1, 2, 3, 5, 6, 7, 4):
                ext = _ext(m)
                nS = ext // P
                sls = _slices(ext)
                qc = wrk.tile([P, NCH, P], bf16, tag="qc", name=f"qc{m}")
                nc.sync.dma_start(
                    qc[:], qtd[:, :, m * P:(m + 1) * P].rearrange("i p q -> p i q"))
                opsum = po.tile([P, C], f32, tag="po", name=f"psO{m}")
                zpart = smallp.tile([P, 4], f32, tag="zpart")
                for si, (off, w) in enumerate(sls):
                    ps = pa.tile([P, 512], f32, tag="pa")
                    for k in range(NCH):
                        nc.tensor.matmul(ps[:, 0:w], qc[:, k, :],
                                         kt[k][:, off:off + w],
                                         start=(k == 0), stop=(k == NCH - 1))
                    if off + w == ext:
                        nc.vector.tensor_tensor(
                            ps[:, w - 256:w], ps[:, w - 256:w],
                            mkt[:, ext - 256:ext], ALU.add)
                    es = wrk.tile([P, 512], bf16, tag="es")
                    nc.scalar.activation(es[:, 0:w], ps[:, 0:w], AF.Exp,
                                         bias=0.0, scale=1.0 / 32.0,
                                         accum_out=zpart[:, si:si + 1])
                    pt = ptr.tile([P, 1024], bf16, tag="ptr")
                    for jj in range(w // P):
                        nc.tensor.transpose(pt[:, jj * P:(jj + 1) * P],
                                            es[:, jj * P:(jj + 1) * P], idnt[:])
                    et = wrk.tile([P, 512], bf16, tag="et")
                    nc.vector.tensor_copy(et[:, 0:w], pt[:, 0:w])
                    for jj in range(w // P):
                        j = off // P + jj
                        for ch in range(2):
                            nc.tensor.matmul(
                                opsum[:, ch * 512:(ch + 1) * 512],
                                et[:, jj * P:(jj + 1) * P],
                                vt[j][:, ch * 512:(ch + 1) * 512],
                                start=(j == 0), stop=(j == nS - 1))
                z = smallp.tile([P, 1], f32, tag="z")
                nc.vector.tensor_reduce(z[:], zpart[:, 0:len(sls)],
                                        axis=mybir.AxisListType.X, op=ALU.add)
                zinv = smallp.tile([P, 1], f32, tag="zinv")
                nc.vector.reciprocal(zinv[:], z[:])
                ysb = wrk.tile([P, C], bf16, tag="osb")
                for ch in range(2):
                    chl = slice(ch * 512, (ch + 1) * 512)
                    nc.scalar.activation(ysb[:, chl], opsum[:, chl], AF.Copy,
                                         bias=0.0, scale=zinv[:])
                    if y_bias:
                        nc.vector.tensor_tensor(ysb[:, chl], ysb[:, chl],
                                                borft[:, chl], ALU.add)
                    nc.sync.dma_start(y[m * P:(m + 1) * P, chl], ysb[:, chl])

    nc.compile()
    return nc


def _zebra_idx(h):
    return np.concatenate(
        [np.arange(P) + (2 * m + h) * P for m in range(NQC)])


SIGMA = np.concatenate([np.arange(0, C, 2), np.arange(1, C, 2)])


def make_host_tables():
    """Half-height tables for the NeoX-pair channel layout."""
    inv_freq = 1.0 / (THETA ** (np.arange(0, C, 2, dtype=np.float64) / C))
    freqs = np.arange(T, dtype=np.float64)[:, None] * inv_freq[None, :]
    cos = np.cos(freqs).T.astype(np.float32)                   # [C/2, T]
    sin = np.sin(freqs).T.astype(np.float32)
    idn = np.eye(P, dtype=np.float32)
    return cos, sin, idn


def _make_mask(h):
    mkv = np.zeros((P, TK), np.float32)
    tri = np.where(np.arange(P)[:, None] >= np.arange(P)[None, :], 0.0, NEG)
    for m in range(NQC):
        base = 256 * m
        g = 2 * m + h
        for bb in range(2):
            kb = 2 * m + bb
            colsl = slice(base + bb * P, base + (bb + 1) * P)
            if kb == g:
                mkv[:, colsl] = tri
            elif kb > g:
                mkv[:, colsl] = NEG
    return mkv


def _q8(a):
    return np.asarray(a, F8).astype(np.float32)


def _hilo(a):
    """(hi, lo, hi/64) fp8 split, lo unscaled, all as F8 arrays."""
    hi = _q8(a)
    lo = (a - hi).astype(F8)
    h64 = (hi / LOSC).astype(F8)
    return hi.astype(F8), lo, h64


def _wsplit(w):
    """(W_hi, 64*W_lo) fp8 split for weights."""
    hi = _q8(w)
    lo = (LOSC * (w - hi)).astype(F8)
    return hi.astype(F8), lo


def _pairpack(a, width):
    """[C, width] -> [NPR, P, 2*width] chunk-pair packing (contraction rows)."""
    a4 = np.ascontiguousarray(a).reshape(NCH, P, width)
    out = np.empty((NPR, P, 2 * width), a.dtype)
    for j in range(NPR):
        out[j, :, :width] = a4[2 * j]
        out[j, :, width:] = a4[2 * j + 1]
    return np.ascontiguousarray(out)


def make_in_maps(x, Wq, bq, Wk, bk, Wv, bv, Wo, bo):
    cos, sin, idn = make_host_tables()
    Wt = (Wv.astype(np.float64) @ Wo.astype(np.float64)).astype(np.float32)
    bo2 = (bv.astype(np.float64) @ Wo.astype(np.float64) + bo).astype(np.float32)

    wk_h, wk_l = _wsplit(Wk[:, SIGMA].astype(np.float32))
    wq_h, wq_l = _wsplit(Wq[:, SIGMA].astype(np.float32))
    wv_h, wv_l = _wsplit(Wt)

    base = {
        "wkh": _pairpack(wk_h, C), "wkl": _pairpack(wk_l, C),
        "wqh": _pairpack(wq_h, C), "wql": _pairpack(wq_l, C),
        "wvh": _pairpack(wv_h, C), "wvl": _pairpack(wv_l, C),
        "bq2": np.ascontiguousarray(
            bq[SIGMA].reshape(NCH, P).T.astype(np.float32)),
        "bk2": np.ascontiguousarray(
            bk[SIGMA].reshape(NCH, P).T.astype(np.float32)),
        "borf": np.ascontiguousarray(
            np.broadcast_to(bo2.reshape(1, C), (P, C))).astype(BF),
        "coskT": np.ascontiguousarray(cos.astype(BF)),
        "sinkT": np.ascontiguousarray(sin.astype(BF)),
        "idn": idn.astype(BF),
    }
    in_maps = []
    xb_cache = {}
    for core in range(8):
        b, h = core // 2, core % 2
        idx = _zebra_idx(h)
        m = dict(base)
        if b not in xb_cache:
            xkT = x[b].T.astype(np.float32)          # [C, TK]
            kh, kl, k64 = _hilo(xkT)
            xb_cache[b] = (_pairpack(kh, TK), _pairpack(kl, TK),
                           _pairpack(k64, TK))
        m["xkh"], m["xkl"], m["xk64"] = xb_cache[b]
        xqT = x[b][idx].T.astype(np.float32)         # [C, TQ]
        qh, ql, q64 = _hilo(xqT)
        m["xqh"] = _pairpack(qh, TQ)
        m["xql"] = _pairpack(ql, TQ)
        m["xq64"] = _pairpack(q64, TQ)
        m["cosqT"] = np.ascontiguousarray(cos[:, idx].astype(BF))
        m["sinqT"] = np.ascontiguousarray(sin[:, idx].astype(BF))
        m["mk"] = _make_mask(h).astype(BF)
        in_maps.append(m)
    return in_maps


_progs = {}


def kernel(x, Wq, bq, Wk, bk, Wv, bv, Wo, bo, _trace=False, _tracedir=None):
    x = np.ascontiguousarray(np.asarray(x, np.float32))
    args = [np.ascontiguousarray(np.asarray(a, np.float32)) for a in
            (Wq, bq, Wk, bk, Wv, bv, Wo, bo)]
    Wo_, bo_, bv_ = args[6], args[7], args[5]
    bor_val = bv_.astype(np.float64) @ Wo_.astype(np.float64) + bo_
    y_bias = bool(np.any(bor_val != 0.0))
    if y_bias not in _progs:
        _progs[y_bias] = build_program(y_bias=y_bias)
    _prog = _progs[y_bias]
    in_maps = make_in_maps(x, *args)
    kw = {}
    if _trace:
        kw = dict(trace=True, trace_cores=[0], tmpdir=_tracedir)
    res = run_bass_kernel_spmd(_prog, in_maps, core_ids=list(range(8)), **kw)
    out = np.empty((B, T, C), np.float32)
    for core in range(8):
        b, h = core // 2, core % 2
        out[b, _zebra_idx(h), :] = res.results[core]["y"].astype(np.float32)
    if _trace:
        kernel._last_results = res
    return out


# revision 53
# speedup vs baseline: 1.3554x; 1.0317x over previous
"""Single-head causal attention with RoPE on 8 trn2 NeuronCores.

B=4, T=2048, C=1024 fp32 in/out; tolerance 2e-2.
Sharding: core c = (batch b = c//2, zebra-half h = c%2).  Query chunks are
interleaved 128-row blocks: core h owns global chunks {2m+h}.  Keys stay in
natural order; chunk m attends keys [0, 256*(m+1)); per-core masks arrive
as data (mk tile).

Key optimizations over the bf16 baseline:
- Wo folded into Wv on the host (W~ = Wv@Wo): y = (P @ (x@W~))/z + bor.
  The entire output projection, its transposes and copies are gone.
- K/V~/Q projections run as fp8(e4m3) DoubleRow matmuls (2 contraction
  chunks per instruction, 0.5 cyc/col) with a 3-pass hi/lo split:
      out = x_hi@W_hi + (x_hi/64)@(64*W_lo) + x_lo@W_hi
  All passes accumulate into one PSUM chain at scale 1 (scales folded
  into host-prepared operands).  Accuracy ~ bf16 (rel err ~3e-3).
- Scores and P@V stay bf16 (fp8 there costs more vector work than it
  saves in PE time and hurts accuracy).
"""

import os
import sys

for _p in ("/opt/trn_rl_repo", "/root/.axon_site/_ro/trn_rl_repo"):
    if os.path.isdir(_p) and _p not in sys.path:
        sys.path.insert(0, _p)

import numpy as np
import ml_dtypes

import concourse.bass as bass
import concourse.bacc as bacc
import concourse.mybir as mybir
from concourse.tile import TileContext
from concourse.bass_utils import run_bass_kernel_spmd

f32 = mybir.dt.float32
bf16 = mybir.dt.bfloat16
f8 = mybir.dt.float8e4
AF = mybir.ActivationFunctionType
ALU = mybir.AluOpType
DR = mybir.MatmulPerfMode.DoubleRow
BF = ml_dtypes.bfloat16
F8 = ml_dtypes.float8_e4m3

B, T, C = 4, 2048, 1024
P = 128
TQ = T // 2           # queries per core
TK = T
NCH = C // P          # 8 channel chunks
NPR = NCH // 2        # 4 contraction chunk-pairs
NQC = TQ // P         # 8 query chunks per core
THETA = 10000.0
NEG = -1.0e9
LOSC = 64.0           # lo-part scale folded into host operands


def _ext(m):
    return 256 * (m + 1)


def _slices(n, step=512):
    out, i = [], 0
    while i < n:
        out.append((i, min(step, n - i)))
        i += step
    return out


def build_program(y_bias=True):
    nc = bacc.Bacc(None, target_bir_lowering=False)

    # x^T chunk-packed: [P, (chunk, t)] -- chunk c occupies cols
    # [c*T, (c+1)*T); DoubleRow pair j = chunks (2j, 2j+1).
    # hi = fp8(x), lo = fp8(x - hi) (unscaled), h64 = fp8(hi/64).
    xkh = nc.dram_tensor("xkh", [P, NCH * TK], f8, kind="ExternalInput")
    xkl = nc.dram_tensor("xkl", [P, NCH * TK], f8, kind="ExternalInput")
    xk64 = nc.dram_tensor("xk64", [P, NCH * TK], f8, kind="ExternalInput")
    xqh = nc.dram_tensor("xqh", [P, NCH * TQ], f8, kind="ExternalInput")
    xql = nc.dram_tensor("xql", [P, NCH * TQ], f8, kind="ExternalInput")
    xq64 = nc.dram_tensor("xq64", [P, NCH * TQ], f8, kind="ExternalInput")
    # weights chunk-packed along input-channel rows: [P, (chunk, cout)]
    wkh = nc.dram_tensor("wkh", [P, NCH * C], f8, kind="ExternalInput")
    wkl = nc.dram_tensor("wkl", [P, NCH * C], f8, kind="ExternalInput")
    wqh = nc.dram_tensor("wqh", [P, NCH * C], f8, kind="ExternalInput")
    wql = nc.dram_tensor("wql", [P, NCH * C], f8, kind="ExternalInput")
    wvh = nc.dram_tensor("wvh", [P, NCH * C], f8, kind="ExternalInput")
    wvl = nc.dram_tensor("wvl", [P, NCH * C], f8, kind="ExternalInput")
    bq2 = nc.dram_tensor("bq2", [P, NCH], f32, kind="ExternalInput")
    bk2 = nc.dram_tensor("bk2", [P, NCH], f32, kind="ExternalInput")
    borf = nc.dram_tensor("borf", [P, C], bf16, kind="ExternalInput")
    # combined cos|sin tables: [:, :T] = cos, [:, T:] = sin
    cskT = nc.dram_tensor("cskT", [C // 2, 2 * TK], bf16, kind="ExternalInput")
    csqT = nc.dram_tensor("csqT", [C // 2, 2 * TQ], bf16, kind="ExternalInput")
    mk = nc.dram_tensor("mk", [P, TK], bf16, kind="ExternalInput")
    idn = nc.dram_tensor("idn", [P, P], bf16, kind="ExternalInput")
    y = nc.dram_tensor("y", [TQ, C], bf16, kind="ExternalOutput")

    def pair2(t):
        return t.rearrange("p (two c) -> p two c", two=2)

    def jv(t, j):
        """[P, NPR*2*w] tile -> [P, 2, w] view of contraction pair j."""
        return t.rearrange("p (j two c) -> p j two c", j=NPR, two=2)[:, j]

    with TileContext(nc) as tc:
        with (
            tc.tile_pool(name="resid", bufs=1) as resid,
            tc.tile_pool(name="dram", bufs=1, space="DRAM") as dpool,
            tc.tile_pool(name="wgt", bufs=1) as wgtp,
            tc.tile_pool(name="xb", bufs=24) as xbp,
            tc.tile_pool(name="xq", bufs=1) as xqp,
            tc.tile_pool(name="rope", bufs=1) as ropep,
            tc.tile_pool(name="wrk", bufs=4) as wrk,
            tc.tile_pool(name="small", bufs=4) as smallp,
            tc.tile_pool(name="psA", bufs=3, space="PSUM") as pa,
            tc.tile_pool(name="psT", bufs=1, space="PSUM") as ptr,
            tc.tile_pool(name="psO", bufs=2, space="PSUM") as po,
        ):
            idnt = resid.tile([P, P], bf16, name="idnt")
            bq2t = resid.tile([P, NCH], f32, name="bq2t")
            bk2t = resid.tile([P, NCH], f32, name="bk2t")

            wkht = wgtp.tile([P, NCH * C], f8, name="wkht")
            wklt = wgtp.tile([P, NCH * C], f8, name="wklt")
            wvht = wgtp.tile([P, NCH * C], f8, name="wvht")
            wvlt = wgtp.tile([P, NCH * C], f8, name="wvlt")
            kt = [resid.tile([P, TK], bf16, name=f"kt{i}") for i in range(NCH)]
            vt = [resid.tile([P, C], bf16, name=f"vt{j}") for j in range(TK // P)]
            qt = [resid.tile([P, TQ], bf16, name=f"qt{i}") for i in range(NCH)]

            # ============ A1+A2: K-proj + RoPE, V~-proj (shared x tiles) ====
            # V~-proj for slice n runs in iteration n+1 (software pipeline)
            # so its weight/x DMAs are never on the PE critical path.
            def v_proj(xh, xl, x64, n):
                for ms in range(4):
                    jg = n * 4 + ms
                    msl = slice(ms * P, (ms + 1) * P)
                    psv = po.tile([P, C], f32, tag="po")
                    for ch in range(2):
                        chl = slice(ch * 512, (ch + 1) * 512)
                        seq = ([(xh, wvht, j) for j in range(NPR)]
                               + [(xl, wvht, j) for j in range(NPR)]
                               + [(x64, wvlt, j) for j in range(NPR)])
                        for s, (xt, wt, j) in enumerate(seq):
                            nc.tensor.matmul(
                                psv[:, chl], jv(xt[:], j)[:, :, msl],
                                jv(wt[:], j)[:, :, chl],
                                start=(s == 0), stop=(s == len(seq) - 1),
                                perf_mode=DR)
                    nc.scalar.activation(vt[jg][:], psv[:], AF.Copy,
                                         bias=0.0, scale=1.0)

            vprev = None
            for n in range(TK // 512):
                sl = slice(n * 512, (n + 1) * 512)

                def xload(src):
                    t = xbp.tile([P, NCH * 512], f8, tag="xb", bufs=6)
                    nc.sync.dma_start(
                        t[:],
                        src.rearrange("p (r t) -> p r t", r=NCH)[:, :, sl])
                    return t

                # DMA issue order tracks the chain's pass dependency order
                # (hi pass, lo pass, 64 pass); weight col-pieces (0,2,1,3)
                # unblock output chunks 0/4 first
                xh = xload(xkh)
                if n == 0:
                    wk4 = wkh.rearrange("p (r c) -> p r c", r=2 * NPR)
                    for kk in (0, 1):
                        ksl = slice(kk * 512, (kk + 1) * 512)
                        nc.sync.dma_start(
                            wkht[:].rearrange("p (r c) -> p r c",
                                              r=2 * NPR)[:, :, ksl],
                            wk4[:, :, ksl])
                    nc.sync.dma_start(bk2t[:], bk2[:])
                    nc.sync.dma_start(bq2t[:], bq2[:])
                xl = xload(xkl)
                if n == 0:
                    wl4 = wkl.rearrange("p (r c) -> p r c", r=2 * NPR)
                    wlt4 = wklt[:].rearrange("p (r c) -> p r c", r=2 * NPR)
                    nc.sync.dma_start(wlt4[:, :, 0:512], wl4[:, :, 0:512])
                x64 = xload(xk64)
                if n == 0:
                    nc.sync.dma_start(wlt4[:, :, 512:1024], wl4[:, :, 512:1024])
                    nc.sync.dma_start(idnt[:], idn[:])
                    nc.sync.dma_start(wvht[:], wvh[:])
                    nc.sync.dma_start(wvlt[:], wvl[:])
                cskn = []
                for i in range(4):
                    t = ropep.tile([P, 1024], bf16, tag="csk", bufs=6)
                    nc.gpsimd.dma_start(
                        t[:], cskT[i * P:(i + 1) * P]
                        .rearrange("p (two t) -> p two t", two=2)[:, :, sl])
                    cskn.append(t)

                def k_pair_rope(i, ri, rj, sl=sl, n=n):
                    """NeoX-pair RoPE: chunks (i, i+4) rotate into kt."""
                    jj = i + 4
                    cs = cskn[i][:, 0:512]
                    sn = cskn[i][:, 512:1024]
                    t1 = wrk.tile([P, 512], bf16, tag="rot", bufs=3)
                    t2 = wrk.tile([P, 512], bf16, tag="rot", bufs=3)
                    nc.vector.tensor_tensor(t1[:], rj[:], sn, ALU.mult)
                    nc.vector.tensor_tensor(t2[:], ri[:], sn, ALU.mult)
                    nc.vector.tensor_tensor(kt[i][:, sl], ri[:], cs,
                                            ALU.mult)
                    nc.vector.tensor_tensor(kt[i][:, sl], kt[i][:, sl], t1[:],
                                            ALU.subtract)
                    nc.vector.tensor_tensor(kt[jj][:, sl], rj[:], cs,
                                            ALU.mult)
                    nc.vector.tensor_tensor(kt[jj][:, sl], kt[jj][:, sl],
                                            t2[:], ALU.add)

                # K-proj in partner order; rope fires when a pair completes
                raw = {}
                for i in (0, 4, 1, 5, 2, 6, 3, 7):
                    ps = pa.tile([P, 512], f32, tag="pa")
                    csl = slice(i * P, (i + 1) * P)
                    seq = ([(wkht, xh, j) for j in range(NPR)]
                           + [(wkht, xl, j) for j in range(NPR)]
                           + [(wklt, x64, j) for j in range(NPR)])
                    for s, (wt, xt, j) in enumerate(seq):
                        nc.tensor.matmul(ps[:], jv(wt[:], j)[:, :, csl],
                                         jv(xt[:], j),
                                         start=(s == 0), stop=(s == len(seq) - 1),
                                         perf_mode=DR)
                    ri = wrk.tile([P, 512], bf16, tag="qraw")
                    nc.scalar.activation(ri[:], ps[:], AF.Identity,
                                         bias=bk2t[:, i:i + 1], scale=1.0)
                    raw[i] = ri
                    if i >= 4:
                        k_pair_rope(i - 4, raw.pop(i - 4), raw.pop(i))
                if n == 3:
                    # Q weights reuse the K weight tiles (K-proj is done
                    # after this iteration's chains read them)
                    nc.sync.dma_start(wkht[:], wqh[:])
                    nc.sync.dma_start(wklt[:], wql[:])
                v_proj(xh, xl, x64, n)

            # ============ A3: Q-proj + RoPE -> qtd ============
            mkt = resid.tile([P, TK], bf16, name="mkt")
            nc.sync.dma_start(mkt[:], mk[:])
            if y_bias:
                borft = resid.tile([P, C], bf16, name="borft")
                nc.sync.dma_start(borft[:], borf[:])
            xqt = {}
            for key, src in (("h", xqh), ("l", xql), ("s", xq64)):
                t = xqp.tile([P, NCH * TQ], f8, name=f"xq{key}")
                nc.sync.dma_start(t[:], src[:])
                xqt[key] = t
            def a3_slice(n):
                sl = slice(n * 512, (n + 1) * 512)
                xh, x64, xl = xqt["h"], xqt["s"], xqt["l"]
                csqn = []
                for i in range(4):
                    t = ropep.tile([P, 1024], bf16, tag="csk", bufs=6)
                    nc.gpsimd.dma_start(
                        t[:], csqT[i * P:(i + 1) * P]
                        .rearrange("p (two t) -> p two t", two=2)[:, :, sl])
                    csqn.append(t)

                def q_pair_rope(i, ri, rj, sl=sl, n=n):
                    jj = i + 4
                    cs = csqn[i][:, 0:512]
                    sn = csqn[i][:, 512:1024]
                    t1 = wrk.tile([P, 512], bf16, tag="rot", bufs=3)
                    t2 = wrk.tile([P, 512], bf16, tag="rot", bufs=3)
                    nc.vector.tensor_tensor(t1[:], rj[:], sn, ALU.mult)
                    nc.vector.tensor_tensor(t2[:], ri[:], sn, ALU.mult)
                    nc.vector.tensor_tensor(qt[i][:, sl], ri[:], cs, ALU.mult)
                    nc.vector.tensor_tensor(qt[i][:, sl], qt[i][:, sl], t1[:],
                                            ALU.subtract)
                    nc.vector.tensor_tensor(qt[jj][:, sl], rj[:], cs, ALU.mult)
                    nc.vector.tensor_tensor(qt[jj][:, sl], qt[jj][:, sl],
                                            t2[:], ALU.add)

                raw = {}
                for i in (0, 4, 1, 5, 2, 6, 3, 7):
                    ps = pa.tile([P, 512], f32, tag="pa")
                    csl = slice(i * P, (i + 1) * P)
                    seq = ([(wkht, xh, j) for j in range(NPR)]
                           + [(wkht, xl, j) for j in range(NPR)]
                           + [(wklt, x64, j) for j in range(NPR)])
                    for s, (wt, xt, j) in enumerate(seq):
                        nc.tensor.matmul(ps[:], jv(wt[:], j)[:, :, csl],
                                         jv(xt[:], j)[:, :, sl],
                                         start=(s == 0), stop=(s == len(seq) - 1),
                                         perf_mode=DR)
                    qraw = wrk.tile([P, 512], bf16, tag="qraw")
                    nc.scalar.activation(qraw[:], ps[:], AF.Identity,
                                         bias=bq2t[:, i:i + 1], scale=1.0)
                    raw[i] = qraw
                    if i >= 4:
                        q_pair_rope(i - 4, raw.pop(i - 4), raw.pop(i))

            # ============ B: attention -> y per q-chunk ============
            # Interleaved with A3: chunks 0-3 only need Q cols [0:512]
            # (A3 slice 0), so they weave between the two Q-proj slices.
            def b_chunk(m):
                mo = m * P
                ext = _ext(m)
                nS = ext // P
                sls = _slices(ext)
                opsum = po.tile([P, C], f32, tag="po", name=f"psO{m}")
                zpart = smallp.tile([P, 4], f32, tag="zpart")
                for si, (off, w) in enumerate(sls):
                    ps = pa.tile([P, 512], f32, tag="pa")
                    for k in range(NCH):
                        nc.tensor.matmul(ps[:, 0:w], qt[k][:, mo:mo + P],
                                         kt[k][:, off:off + w],
                                         start=(k == 0), stop=(k == NCH - 1))
                    if off + w == ext:
                        nc.vector.tensor_tensor(
                            ps[:, w - 256:w], ps[:, w - 256:w],
                            mkt[:, ext - 256:ext], ALU.add)
                    es = wrk.tile([P, 512], bf16, tag="es", bufs=2)
                    nc.scalar.activation(es[:, 0:w], ps[:, 0:w], AF.Exp,
                                         bias=0.0, scale=1.0 / 32.0,
                                         accum_out=zpart[:, si:si + 1])
                    pt = ptr.tile([P, 1024], bf16, tag="ptr")
                    for jj in range(w // P):
                        nc.tensor.transpose(pt[:, jj * P:(jj + 1) * P],
                                            es[:, jj * P:(jj + 1) * P],
                                            idnt[:])
                    et = wrk.tile([P, 512], bf16, tag="et", bufs=2)
                    nc.vector.tensor_copy(et[:, 0:w], pt[:, 0:w])
                    for jj in range(w // P):
                        j = off // P + jj
                        for ch in range(2):
                            nc.tensor.matmul(
                                opsum[:, ch * 512:(ch + 1) * 512],
                                et[:, jj * P:(jj + 1) * P],
                                vt[j][:, ch * 512:(ch + 1) * 512],
                                start=(j == 0), stop=(j == nS - 1))
                z = smallp.tile([P, 1], f32, tag="z")
                nc.vector.tensor_reduce(z[:], zpart[:, 0:len(sls)],
                                        axis=mybir.AxisListType.X, op=ALU.add)
                zinv = smallp.tile([P, 1], f32, tag="zinv")
                nc.vector.reciprocal(zinv[:], z[:])
                ysb = wrk.tile([P, C], bf16, tag="osb", bufs=2)
                for ch in range(2):
                    chl = slice(ch * 512, (ch + 1) * 512)
                    nc.scalar.activation(ysb[:, chl], opsum[:, chl], AF.Copy,
                                         bias=0.0, scale=zinv[:])
                    if y_bias:
                        nc.vector.tensor_tensor(ysb[:, chl], ysb[:, chl],
                                                borft[:, chl], ALU.add)
                    nc.sync.dma_start(y[m * P:(m + 1) * P, chl],
                                      ysb[:, chl])

            a3_slice(0)
            for m in (0, 1, 2, 3):
                b_chunk(m)
            a3_slice(1)
            for m in (5, 6, 7, 4):
                b_chunk(m)

    nc.compile()
    return nc


def _zebra_idx(h):
    return np.concatenate(
        [np.arange(P) + (2 * m + h) * P for m in range(NQC)])


SIGMA = np.concatenate([np.arange(0, C, 2), np.arange(1, C, 2)])


def make_host_tables():
    """Half-height tables for the NeoX-pair channel layout."""
    inv_freq = 1.0 / (THETA ** (np.arange(0, C, 2, dtype=np.float64) / C))
    freqs = np.arange(T, dtype=np.float64)[:, None] * inv_freq[None, :]
    cos = np.cos(freqs).T.astype(np.float32)                   # [C/2, T]
    sin = np.sin(freqs).T.astype(np.float32)
    idn = np.eye(P, dtype=np.float32)
    return cos, sin, idn


def _make_mask(h):
    mkv = np.zeros((P, TK), np.float32)
    tri = np.where(np.arange(P)[:, None] >= np.arange(P)[None, :], 0.0, NEG)
    for m in range(NQC):
        base = 256 * m
        g = 2 * m + h
        for bb in range(2):
            kb = 2 * m + bb
            colsl = slice(base + bb * P, base + (bb + 1) * P)
            if kb == g:
                mkv[:, colsl] = tri
            elif kb > g:
                mkv[:, colsl] = NEG
    return mkv


def _q8(a):
    return np.asarray(a, F8).astype(np.float32)


def _hilo(a):
    """(hi, lo, hi/64) fp8 split, lo unscaled, all as F8 arrays."""
    hi = _q8(a)
    lo = (a - hi).astype(F8)
    h64 = (hi / LOSC).astype(F8)
    return hi.astype(F8), lo, h64


def _wsplit(w):
    """(W_hi, 64*W_lo) fp8 split for weights."""
    hi = _q8(w)
    lo = (LOSC * (w - hi)).astype(F8)
    return hi.astype(F8), lo


def _chunkpack(a, width):
    """[C, width] -> [P, NCH*width]: region r holds contraction-row chunk r."""
    a4 = np.ascontiguousarray(a).reshape(NCH, P, width)
    return np.ascontiguousarray(
        a4.transpose(1, 0, 2).reshape(P, NCH * width))


def make_in_maps(x, Wq, bq, Wk, bk, Wv, bv, Wo, bo):
    cos, sin, idn = make_host_tables()
    Wt = (Wv.astype(np.float64) @ Wo.astype(np.float64)).astype(np.float32)
    bo2 = (bv.astype(np.float64) @ Wo.astype(np.float64) + bo).astype(np.float32)

    wk_h, wk_l = _wsplit(Wk[:, SIGMA].astype(np.float32))
    wq_h, wq_l = _wsplit(Wq[:, SIGMA].astype(np.float32))
    wv_h, wv_l = _wsplit(Wt)

    base = {
        "wkh": _chunkpack(wk_h, C), "wkl": _chunkpack(wk_l, C),
        "wqh": _chunkpack(wq_h, C), "wql": _chunkpack(wq_l, C),
        "wvh": _chunkpack(wv_h, C), "wvl": _chunkpack(wv_l, C),
        "bq2": np.ascontiguousarray(
            bq[SIGMA].reshape(NCH, P).T.astype(np.float32)),
        "bk2": np.ascontiguousarray(
            bk[SIGMA].reshape(NCH, P).T.astype(np.float32)),
        "borf": np.ascontiguousarray(
            np.broadcast_to(bo2.reshape(1, C), (P, C))).astype(BF),
        "cskT": np.ascontiguousarray(
            np.concatenate([cos, sin], axis=1).astype(BF)),
        "idn": idn.astype(BF),
    }
    in_maps = []
    xb_cache = {}
    for core in range(8):
        b, h = core // 2, core % 2
        idx = _zebra_idx(h)
        m = dict(base)
        if b not in xb_cache:
            xkT = x[b].T.astype(np.float32)          # [C, TK]
            kh, kl, k64 = _hilo(xkT)
            xb_cache[b] = (_chunkpack(kh, TK), _chunkpack(kl, TK),
                           _chunkpack(k64, TK))
        m["xkh"], m["xkl"], m["xk64"] = xb_cache[b]
        xqT = x[b][idx].T.astype(np.float32)         # [C, TQ]
        qh, ql, q64 = _hilo(xqT)
        m["xqh"] = _chunkpack(qh, TQ)
        m["xql"] = _chunkpack(ql, TQ)
        m["xq64"] = _chunkpack(q64, TQ)
        m["csqT"] = np.ascontiguousarray(
            np.concatenate([cos[:, idx], sin[:, idx]], axis=1).astype(BF))
        m["mk"] = _make_mask(h).astype(BF)
        in_maps.append(m)
    return in_maps


_progs = {}


def kernel(x, Wq, bq, Wk, bk, Wv, bv, Wo, bo, _trace=False, _tracedir=None):
    x = np.ascontiguousarray(np.asarray(x, np.float32))
    args = [np.ascontiguousarray(np.asarray(a, np.float32)) for a in
            (Wq, bq, Wk, bk, Wv, bv, Wo, bo)]
    Wo_, bo_, bv_ = args[6], args[7], args[5]
    bor_val = bv_.astype(np.float64) @ Wo_.astype(np.float64) + bo_
    y_bias = bool(np.any(bor_val != 0.0))
    if y_bias not in _progs:
        _progs[y_bias] = build_program(y_bias=y_bias)
    _prog = _progs[y_bias]
    in_maps = make_in_maps(x, *args)
    kw = {}
    if _trace:
        kw = dict(trace=True, trace_cores=[0], tmpdir=_tracedir)
    res = run_bass_kernel_spmd(_prog, in_maps, core_ids=list(range(8)), **kw)
    out = np.empty((B, T, C), np.float32)
    for core in range(8):
        b, h = core // 2, core % 2
        out[b, _zebra_idx(h), :] = res.results[core]["y"].astype(np.float32)
    if _trace:
        kernel._last_results = res
    return out
